# revision 45
# baseline (speedup 1.0000x reference)
"""DeepSeek-V3 MLA attention on 8 TRN2 NeuronCores (Bass/Tile) — v4.

v3 (norm folded into stage-2 copies, kt-outer chunked stage 1, deferred
softmax-denominator, hoisted attention loads) plus a latent-AllGather:
instead of all-to-all'ing K and V per head, each core AllGathers the
normalized kv-latent + roped k_pe (576 x 512 bf16 = 0.59 MB payload) and
recomputes its 2 heads' K/V from the gathered latent on the attention
side. The K/V GEMM work is sharding-invariant, so PE cost is unchanged,
while HBM traffic drops ~12 MB/core (K/V A2A roundtrips + the kpe
broadcast disappear, and wkvb shrinks to the core's 2-head slice).
"""
from collections import deque
from contextlib import ExitStack

import numpy as np
import ml_dtypes

import concourse.bass_isa as bass_isa
import concourse.mybir as mybir
import concourse.tile as tile
from concourse import bacc
from concourse.bass_utils import run_bass_kernel_spmd

BF16NP = ml_dtypes.bfloat16
SCALE = 192 ** -0.5

dt = mybir.dt
F32, BF16 = dt.float32, dt.bfloat16

P = 128
NC_ = 8
LR = 512               # local rows per core
NH = 16
Q_LORA, KV_LORA = 1536, 512
NLAT = Q_LORA + KV_LORA + 64    # 2112
NOPE, ROPE, VH = 128, 64, 128
EPS = 1e-6
B, S = 2, 2048
R = B * S

# latent-gather buffer: [4x128 normalized kv-lat kt-tiles | 64 roped kpe]
GKPE_O = 512 * 512
SHG = 576 * 512
# Q all-to-alls (per parity): per-dest [Qr 64x512 | Qn 128x512]
QR_O = 0
QN_O = 64 * 512
SHQ = 192 * 512
SH2 = 128 * 512           # O send per hl

# stage-1 pt chunks (kv-latent first so the gather can fire early)
CHUNKS = [[12, 13, 14, 15, 16], [0, 1, 2, 3, 4, 5], [6, 7, 8, 9, 10, 11]]

# wqb host column layout: [rope_e 512 | rope_o 512 | even nope 1024 | odd 1024]
def _qcol(pt):
    if 16 <= pt < 20:
        return (pt - 16) * 128
    if 20 <= pt < 24:
        return 512 + (pt - 20) * 128
    if pt % 2 == 0:
        return 1024 + (pt // 2) * 128
    return 2048 + (pt // 2) * 128


def _blk(dram, j, off, rows, width=512):
    """[rows, width] view at element offset `off` of flat shard j."""
    return dram[j, off:off + rows * width].rearrange("(p c) -> p c", c=width)


def build_kernel(reps: int = 1, debug: bool = False, loopback: bool = False):
    nc = bacc.Bacc(None, target_bir_lowering=False, debug=False)

    XT = nc.dram_tensor("xt", [2048, LR], BF16, kind="ExternalInput")
    WA = nc.dram_tensor("wa", [2048, NLAT], BF16, kind="ExternalInput")
    WQB = nc.dram_tensor("wqb", [Q_LORA, NH * 192], BF16, kind="ExternalInput")
    # per-core slice: this core's 2 heads, cols [K 128 | V 128] x 2
    WKVB = nc.dram_tensor("wkvb", [KV_LORA, 512], BF16, kind="ExternalInput")
    WO = nc.dram_tensor("wo", [2048, 2048], BF16, kind="ExternalInput")
    COST = nc.dram_tensor("cost", [P, LR], F32, kind="ExternalInput")
    SINT = nc.dram_tensor("sint", [P, LR], F32, kind="ExternalInput")
    MASKS = nc.dram_tensor("masks", [P, 4 * 512], BF16, kind="ExternalInput")
    OUT = nc.dram_tensor("out", [LR, 2048], BF16, kind="ExternalOutput")

    SENDG = nc.dram_tensor("sendg", [SHG], BF16, kind="Internal")
    RECVG = nc.dram_tensor("recvg", [NC_, SHG], BF16, kind="Internal")
    SENDQE = nc.dram_tensor("sendqe", [NC_, SHQ], BF16, kind="Internal")
    RECVQE = nc.dram_tensor("recvqe", [NC_, SHQ], BF16, kind="Internal")
    SENDQO = nc.dram_tensor("sendqo", [NC_, SHQ], BF16, kind="Internal")
    RECVQO = nc.dram_tensor("recvqo", [NC_, SHQ], BF16, kind="Internal")
    SEND2 = [nc.dram_tensor(f"send2{h}", [NC_, SH2], BF16, kind="Internal")
             for h in range(2)]
    RECV2 = [nc.dram_tensor(f"recv2{h}", [NC_, SH2], BF16, kind="Internal")
             for h in range(2)]

    with tile.TileContext(nc) as tc, ExitStack() as octx:
        consts = octx.enter_context(tc.tile_pool(name="consts", bufs=1))
        ones_bf = consts.tile([P, 1], BF16)
        nc.vector.memset(ones_bf, 1.0)
        eps_t = consts.tile([1, 1], F32)
        nc.vector.memset(eps_t, EPS)
        cos_sb = consts.tile([P, LR], F32)
        sin_sb = consts.tile([P, LR], F32)
        masks = consts.tile([P, 4, 512], BF16)
        nc.gpsimd.dma_start(out=cos_sb, in_=COST[:, :])
        nc.gpsimd.dma_start(out=sin_sb, in_=SINT[:, :])
        nc.gpsimd.dma_start(
            out=masks, in_=MASKS[:, :].rearrange("p (m c) -> p m c", c=512))
        cst = dict(ones_bf=ones_bf, cos=cos_sb, sin=sin_sb, eps=eps_t,
                   masks=masks)

        for rep in range(reps):
            _one_rep(nc, tc, rep, XT, WA, WQB, WKVB, WO, OUT,
                     SENDG, RECVG, SENDQE, RECVQE, SENDQO, RECVQO,
                     SEND2, RECV2, cst, loopback)
    nc.finalize()
    return nc


def _one_rep(nc, tc, rep, XT, WA, WQB, WKVB, WO, OUT,
             SENDG, RECVG, SENDQE, RECVQE, SENDQO, RECVQO,
             SEND2, RECV2, cst, loopback=False):
    cos_sb, sin_sb = cst["cos"], cst["sin"]
    ones_bf, eps_t = cst["ones_bf"], cst["eps"]
    masks = cst["masks"]

    def _a2a(send, recv):
        if loopback:
            nc.sync.dma_start(out=recv[:, :], in_=send[:, :])
        else:
            nc.gpsimd.collective_compute(
                "AllToAll", mybir.AluOpType.bypass,
                ins=[send[:, :]], outs=[recv[:, :]],
                replica_groups=[list(range(NC_))])

    def _gather():
        if loopback:
            for j in range(NC_):
                nc.sync.dma_start(out=RECVG[j, :], in_=SENDG[:])
        else:
            nc.gpsimd.collective_compute(
                "AllGather", mybir.AluOpType.bypass,
                ins=[SENDG[:]], outs=[RECVG[:, :]],
                replica_groups=[list(range(NC_))])

    with ExitStack() as ctx:
      # wkvb (this core's 2-head slice) is consumed by the attention-side
      # K/V recompute, so it lives at rep scope, past the projection pools
      kvp = ctx.enter_context(tc.tile_pool(name=f"kvp{rep}", bufs=1))
      wkvb_sb = kvp.tile([P, 4, 512], BF16)
      wkvb_v = WKVB[:, :].rearrange("(kt p) n -> p kt n", p=P)
      with ExitStack() as pctx:
        s1out = pctx.enter_context(tc.tile_pool(name=f"s1out{rep}", bufs=1))
        latt = s1out.tile([P, 17, 512], BF16)     # unnormed lat^T tiles
        latn = s1out.tile([P, 4, 512], BF16)      # normalized kv-lat tiles
        kpe_sb = s1out.tile([64, 512], BF16)      # roped k_pe^T (local rows)
        rq = s1out.tile([1, 512], F32)
        rkv = s1out.tile([1, 512], F32)
        rq_b = s1out.tile([P, 512], F32)
        rkv_b = s1out.tile([P, 512], F32)
        wbp = pctx.enter_context(tc.tile_pool(name=f"wb{rep}", bufs=1))
        NQA = 6
        wqb_a = wbp.tile([P, NQA, NH * 192], BF16)
        wqb_v = WQB[:, :].rearrange("(kt p) n -> p kt n", p=P)

        # ---------------- Stage 1: kt-outer pt-chunks + latent gather ------
        with ExitStack() as sctx:
            wap = sctx.enter_context(tc.tile_pool(name=f"wa{rep}", bufs=1))
            pp1 = sctx.enter_context(tc.tile_pool(name=f"ps1{rep}", bufs=6, space="PSUM"))
            ppq = sctx.enter_context(tc.tile_pool(name=f"psq{rep}", bufs=1, space="PSUM"))
            sqp = sctx.enter_context(tc.tile_pool(name=f"sq{rep}", bufs=6))
            nrm = sctx.enter_context(tc.tile_pool(name=f"nrm{rep}", bufs=1))

            wa_sb = wap.tile([P, 16, NLAT], BF16)
            xt_sb = wap.tile([P, 16, LR], BF16)
            wa_v = WA[:, :].rearrange("(kt p) n -> p kt n", p=P)
            xt_v = XT[:, :].rearrange("(kt p) n -> p kt n", p=P)
            # kv-latent (+k_pe) columns stream first, kt-interleaved with xt
            for kt in range(16):
                nc.sync.dma_start(out=xt_sb[:, kt:kt + 1, :],
                                  in_=xt_v[:, kt:kt + 1, :])
                nc.sync.dma_start(out=wa_sb[:, kt, 1536:2112],
                                  in_=wa_v[:, kt, 1536:2112])
            for c0, c1 in ((0, 384), (384, 768), (768, 1152), (1152, 1536)):
                for kt in range(0, 16, 2):
                    nc.sync.dma_start(out=wa_sb[:, kt:kt + 2, c0:c1],
                                      in_=wa_v[:, kt:kt + 2, c0:c1])
            nc.sync.dma_start(out=wqb_a[:, :, :], in_=wqb_v[:, 0:NQA, :])
            nc.sync.dma_start(out=wkvb_sb[:, :, :], in_=wkvb_v[:, 0:4, :])

            ps_ssq_q = ppq.tile([1, 512], F32)
            ps_ssq_kv = ppq.tile([1, 512], F32)

            def ssq_mm(pt):
                def emit(sq_t):
                    if pt < 12:
                        nc.tensor.matmul(ps_ssq_q, lhsT=ones_bf, rhs=sq_t,
                                         start=(pt == 0), stop=(pt == 11))
                    else:
                        nc.tensor.matmul(ps_ssq_kv, lhsT=ones_bf, rhs=sq_t,
                                         start=(pt == 12), stop=(pt == 15))
                return emit

            def rkv_chain():
                nc.scalar.activation(rkv, ps_ssq_kv,
                                     mybir.ActivationFunctionType.Sqrt,
                                     bias=eps_t, scale=1.0 / KV_LORA)
                nc.vector.reciprocal(rkv, rkv)
                nc.gpsimd.partition_broadcast(rkv_b, rkv)

            def rq_chain():
                nc.scalar.activation(rq, ps_ssq_q,
                                     mybir.ActivationFunctionType.Sqrt,
                                     bias=eps_t, scale=1.0 / Q_LORA)
                nc.vector.reciprocal(rq, rq)
                nc.gpsimd.partition_broadcast(rq_b, rq)

            def latn_mul(i):
                # normalized kv-latent tile + its gather send
                nc.vector.tensor_mul(latn[:, i, :], latt[:, 12 + i, :], rkv_b)
                nc.scalar.dma_start(
                    out=SENDG[i * 128 * 512:(i + 1) * 128 * 512].rearrange(
                        "(p c) -> p c", c=512),
                    in_=latn[:, i, :])

            fillers = deque()

            def run_chunk(chunk):
                tiles = {pt: pp1.tile([P, 512], F32, tag="s1", name=f"s1ps{pt}")
                         for pt in chunk}
                for kt in range(16):
                    for pt in chunk:
                        pw = 128 if pt < 16 else 64
                        nc.tensor.matmul(
                            tiles[pt][:pw, :],
                            lhsT=wa_sb[:, kt, pt * 128:pt * 128 + pw],
                            rhs=xt_sb[:, kt, :],
                            start=(kt == 0), stop=(kt == 15))
                    if kt >= 1 and fillers:
                        fillers.popleft()()
                return tiles

            def drain_chunk(chunk, tiles):
                sqs = []
                for i, pt in enumerate(chunk):
                    pw = 128 if pt < 16 else 64
                    if i % 2 == 0:
                        nc.scalar.copy(latt[:pw, pt, :], tiles[pt][:pw, :])
                    else:
                        nc.vector.tensor_copy(latt[:pw, pt, :], tiles[pt][:pw, :])
                    if pt < 16:
                        sq_t = sqp.tile([P, 512], BF16, tag="sq")
                        eng = nc.gpsimd if i % 2 == 0 else nc.vector
                        eng.tensor_mul(sq_t, latt[:, pt, :], latt[:, pt, :])
                        sqs.append((pt, sq_t))
                return sqs

            # C1: kv-latent pt 12-16 (k_pe rows drain here, so the rope
            # and its gather send fire a full chunk earlier)
            tiles = run_chunk(CHUNKS[0])
            sqs = drain_chunk(CHUNKS[0], tiles)

            kp = nrm.tile([32, 4, 512], BF16, tag="krope")
            xo_c = nrm.tile([32, 512], BF16, tag="kxo")
            nc.scalar.dma_start(out=xo_c, in_=latt[32:64, 16, :])
            xe = latt[0:32, 16, :]
            c32, s32 = cos_sb[0:32, :], sin_sb[0:32, :]
            nc.vector.tensor_mul(kp[:, 0, :], xe, c32)
            nc.vector.tensor_mul(kp[:, 1, :], xe, s32)
            nc.vector.tensor_mul(kp[:, 2, :], xo_c, s32)
            nc.vector.tensor_mul(kp[:, 3, :], xo_c, c32)
            nc.vector.tensor_sub(kpe_sb[0:32, :], kp[:, 0, :], kp[:, 2, :])
            yi = nrm.tile([32, 512], BF16, tag="kyi")
            nc.vector.tensor_add(yi, kp[:, 1, :], kp[:, 3, :])
            nc.scalar.dma_start(out=kpe_sb[32:64, :], in_=yi)
            nc.scalar.dma_start(
                out=SENDG[GKPE_O:GKPE_O + 64 * 512].rearrange(
                    "(p c) -> p c", c=512),
                in_=kpe_sb)

            # C2 fillers: kv ssq, rkv chain (must precede the latn muls on
            # the DVE queue), then the normalized-latent tiles + sends
            for pt, sq_t in sqs:
                fillers.append((lambda e=ssq_mm(pt), s=sq_t: e(s)))
            fillers.append(rkv_chain)
            for i in range(4):
                fillers.append((lambda i=i: latn_mul(i)))

            tiles = run_chunk(CHUNKS[1])        # pt 0-5
            sqs2 = drain_chunk(CHUNKS[1], tiles)
            # latent + kpe gather: fires while chunk 3 still runs
            _gather()

            for pt, sq_t in sqs2:
                fillers.append((lambda e=ssq_mm(pt), s=sq_t: e(s)))
            tiles = run_chunk(CHUNKS[2])        # pt 6-11
            while fillers:
                fillers.popleft()()
            sqs = drain_chunk(CHUNKS[2], tiles)
            for pt, sq_t in sqs:
                ssq_mm(pt)(sq_t)
            rq_chain()

        # ---------------- Stage 2: Q projections + per-parity A2As ---------
        s2out = pctx.enter_context(tc.tile_pool(name=f"s2out{rep}", bufs=1))
        with ExitStack() as sctx:
            pp3 = sctx.enter_context(tc.tile_pool(name=f"ps3{rep}", bufs=6, space="PSUM"))
            rp = sctx.enter_context(tc.tile_pool(name=f"qrope{rep}", bufs=2))
            wqb_b = s2out.tile([P, 12 - NQA, NH * 192], BF16)
            for kt in range(NQA, 12):
                nc.sync.dma_start(out=wqb_b[:, kt - NQA, :], in_=wqb_v[:, kt, :])

            def wqb_t(kt):
                return wqb_a[:, kt, :] if kt < NQA else wqb_b[:, kt - NQA, :]

            qt_sb = s2out.tile([P, 24, 512], BF16)

            def q_tiles(tiles_):
                for pt in tiles_:
                    ps = pp3.tile([P, 512], F32)
                    c0 = _qcol(pt)
                    for kt in range(12):
                        nc.tensor.matmul(
                            ps, lhsT=wqb_t(kt)[:, c0:c0 + 128],
                            rhs=latt[:, kt, :], start=(kt == 0), stop=(kt == 11))
                    nc.vector.tensor_mul(qt_sb[:, pt, :], ps, rq_b)

            def send_qr(par, SEND):
                for j in range(NC_):
                    h = 2 * j + par
                    pe = (h % 4) * 32
                    nc.scalar.dma_start(out=_blk(SEND, j, QR_O, 32),
                                        in_=qt_sb[pe:pe + 32, 16 + h // 4, :])
                    nc.scalar.dma_start(out=_blk(SEND, j, QR_O + 32 * 512, 32),
                                        in_=qt_sb[pe:pe + 32, 20 + h // 4, :])

            # even-parity Q: rope tiles, rotation, then per-dest nope sends
            q_tiles(range(16, 24))      # all rope tiles
            for j in range(4):
                et = qt_sb[:, 16 + j, :]
                ot = qt_sb[:, 20 + j, :]
                t = rp.tile([P, 4, 512], BF16, tag="qr")
                nc.vector.tensor_mul(t[:, 0, :], et, cos_sb)
                nc.vector.tensor_mul(t[:, 1, :], et, sin_sb)
                nc.vector.tensor_mul(t[:, 2, :], ot, sin_sb)
                nc.vector.tensor_mul(t[:, 3, :], ot, cos_sb)
                nc.vector.tensor_sub(et, t[:, 0, :], t[:, 2, :])
                nc.vector.tensor_add(ot, t[:, 1, :], t[:, 3, :])
            send_qr(0, SENDQE)
            for j in range(NC_):
                q_tiles([2 * j])
                nc.scalar.dma_start(out=_blk(SENDQE, j, QN_O, 128),
                                    in_=qt_sb[:, 2 * j, :])
            _a2a(SENDQE, RECVQE)

            # odd-parity Q
            q_tiles(range(1, 16, 2))
            send_qr(1, SENDQO)
            for j in range(NC_):
                nc.scalar.dma_start(out=_blk(SENDQO, j, QN_O, 128),
                                    in_=qt_sb[:, 2 * j + 1, :])
            _a2a(SENDQO, RECVQO)
      # projection pools freed here
      if True:
        # ---------------- Stage 4: attention (hl outer, b inner) -----------
        wop = ctx.enter_context(tc.tile_pool(name=f"wo{rep}", bufs=1))
        # gathered latent + kpe ride the SWDGE ring ahead of the wo
        # prefetch: they start the moment the AllGather lands, and the
        # SP ring stays clear for the Q recv loads
        lat_all = wop.tile([P, 4, 4096], BF16)
        kpe_all = wop.tile([64, 8, 512], BF16)
        for j in range(NC_):
            nc.gpsimd.dma_start(
                out=lat_all[:, :, j * 512:(j + 1) * 512],
                in_=RECVG[j, 0:GKPE_O].rearrange(
                    "(kt p c) -> p kt c", p=128, c=512))
            nc.gpsimd.dma_start(
                out=kpe_all[:, j, :],
                in_=RECVG[j, GKPE_O:GKPE_O + 64 * 512].rearrange(
                    "(p c) -> p c", c=512))
        wo_sb = wop.tile([P, 16, 2048], BF16)
        wo_v = WO[:, :].rearrange("(kt p) n -> p kt n", p=P)
        for kt in range(16):
            nc.gpsimd.dma_start(out=wo_sb[:, kt, :], in_=wo_v[:, kt, :])
        otf = wop.tile([P, 16, 512], BF16)

        with ExitStack() as sctx:
            asm = sctx.enter_context(tc.tile_pool(name=f"asm{rep}", bufs=2))
            ptp = sctx.enter_context(tc.tile_pool(name=f"pt{rep}", bufs=6))
            ppS = sctx.enter_context(tc.tile_pool(name=f"psS{rep}", bufs=4, space="PSUM"))
            ppO = sctx.enter_context(tc.tile_pool(name=f"psO{rep}", bufs=4, space="PSUM"))
            sml = sctx.enter_context(tc.tile_pool(name=f"sml{rep}", bufs=2))
            otp = sctx.enter_context(tc.tile_pool(name=f"ot{rep}", bufs=2))

            pending = [None]

            def flush_pending():
                if pending[0] is not None:
                    pending[0]()
                    pending[0] = None

            for hl in range(2):
                RECVQ = RECVQE if hl == 0 else RECVQO
                ot_sb = otp.tile([P, 4096], BF16, tag="ot")
                for b in range(B):
                    # Q loads for this (hl, b); K/V recomputed from the
                    # gathered latent while these DMAs are in flight
                    qtn = asm.tile([P, 4, 512], BF16, tag="qtn",
                                   name=f"qtn{hl}{b}")
                    qtr = asm.tile([64, 4, 512], BF16, tag="qtr",
                                   name=f"qtr{hl}{b}")
                    s0 = 4 * b
                    nc.sync.dma_start(
                        out=qtr,
                        in_=RECVQ[s0:s0 + 4, QR_O:QR_O + 64 * 512].rearrange(
                            "s (p c) -> p s c", c=512))
                    nc.sync.dma_start(
                        out=qtn,
                        in_=RECVQ[s0:s0 + 4, QN_O:QN_O + 128 * 512].rearrange(
                            "s (p c) -> p s c", c=512))
                    ktn = asm.tile([P, 4, 512], BF16, tag="ktn",
                                   name=f"ktn{hl}{b}")
                    vt = asm.tile([P, 16, 128], BF16, tag="vt",
                                  name=f"vt{hl}{b}")
                    for kb in range(4):
                        ps = ppS.tile([P, 512], F32, tag="psS", name=f"kps{hl}{b}{kb}")
                        for kt in range(4):
                            nc.tensor.matmul(
                                ps,
                                lhsT=wkvb_sb[:, kt, hl * 256:hl * 256 + 128],
                                rhs=lat_all[:, kt,
                                            (4 * b + kb) * 512:(4 * b + kb + 1) * 512],
                                start=(kt == 0), stop=(kt == 3))
                        if kb == 1:
                            flush_pending()
                        eng = nc.vector if kb % 2 == 0 else nc.scalar
                        if kb % 2 == 0:
                            nc.vector.tensor_copy(ktn[:, kb, :], ps)
                        else:
                            nc.scalar.copy(ktn[:, kb, :], ps)
                    for g in range(4):
                        ps = ppS.tile([P, 512], F32, tag="psS", name=f"vps{hl}{b}{g}")
                        for i in range(4):
                            tb = 4 * g + i
                            for kt in range(4):
                                nc.tensor.matmul(
                                    ps[:, i * 128:(i + 1) * 128],
                                    lhsT=lat_all[:, kt,
                                                 (b * 16 + tb) * 128:(b * 16 + tb + 1) * 128],
                                    rhs=wkvb_sb[:, kt,
                                                hl * 256 + 128:hl * 256 + 256],
                                    start=(kt == 0), stop=(kt == 3))
                        if g % 2 == 0:
                            nc.vector.tensor_copy(vt[:, 4 * g:4 * g + 4, :], ps)
                        else:
                            nc.scalar.copy(vt[:, 4 * g:4 * g + 4, :], ps)
                    for qg in range(4):
                        psO = ppO.tile([P, 512], F32)
                        nkt = 4 * qg + 4
                        dacc_a = sml.tile([P, 512], BF16, tag="dacca")
                        prevs = []
                        for kt in range(nkt):
                            m = kt - 4 * qg
                            # dead left columns of diagonal tiles are skipped
                            # everywhere: dacc is initialized full at kt==0 and
                            # the masked region contributes exactly zero
                            lo = 128 * m if m > 0 else 0
                            cs = slice(lo, 512)
                            psS = ppS.tile([P, 512], F32, tag="psS")
                            nc.tensor.matmul(
                                psS[:, cs],
                                lhsT=ktn[:, kt // 4, (kt % 4) * 128:(kt % 4 + 1) * 128],
                                rhs=qtn[:, qg, cs], start=True, stop=False)
                            nc.tensor.matmul(
                                psS[:, cs],
                                lhsT=kpe_all[:, 4 * b + kt // 4, (kt % 4) * 128:(kt % 4 + 1) * 128],
                                rhs=qtr[:, qg, cs], start=False, stop=True)
                            if kt == 2:
                                flush_pending()
                            if kt >= 2:
                                # PV runs two tiles behind its exp so the PE
                                # never waits on the Act-engine latency
                                ppt, pcs = prevs[kt - 2]
                                nc.tensor.matmul(psO[:, pcs], lhsT=vt[:, kt - 2, :],
                                                 rhs=ppt[:, pcs],
                                                 start=(kt == 2), stop=False)
                            pt_t = ptp.tile([P, 512], BF16, tag="pt")
                            nc.scalar.activation(pt_t[:, cs], psS[:, cs],
                                                 mybir.ActivationFunctionType.Exp)
                            if m >= 0:
                                nc.vector.tensor_mul(pt_t[:, cs], pt_t[:, cs],
                                                     masks[:, m, cs])
                            if kt == 0:
                                nc.vector.tensor_copy(dacc_a, pt_t)
                            else:
                                nc.vector.tensor_add(dacc_a[:, cs], dacc_a[:, cs],
                                                     pt_t[:, cs])
                            prevs.append((pt_t, cs))
                        for kt_t in (nkt - 2, nkt - 1):
                            ppt, pcs = prevs[kt_t]
                            nc.tensor.matmul(psO[:, pcs], lhsT=vt[:, kt_t, :],
                                             rhs=ppt[:, pcs],
                                             start=False, stop=(kt_t == nkt - 1))

                        den_b = sml.tile([P, 512], F32, tag="den")
                        nc.gpsimd.partition_all_reduce(
                            den_b, dacc_a, 128, bass_isa.ReduceOp.add)

                        def fin(psO=psO, den=den_b, b=b, qg=qg, ot_sb=ot_sb):
                            rdb = sml.tile([P, 512], F32, tag="rdb")
                            nc.vector.reciprocal(rdb, den)
                            nc.vector.tensor_mul(
                                ot_sb[:, b * 2048 + qg * 512:b * 2048 + (qg + 1) * 512],
                                psO, rdb)
                        flush_pending()
                        pending[0] = fin
                # ship this head, overlap with next head's attention
                flush_pending()
                nc.scalar.dma_start(
                    out=SEND2[hl][:, :].rearrange("j (p c) -> p j c", c=512),
                    in_=ot_sb[:, :].rearrange("p (j c) -> p j c", c=512))
                _a2a(SEND2[hl], RECV2[hl])
                nc.sync.dma_start(
                    out=otf[:, hl:16:2, :],
                    in_=RECV2[hl][:, :].rearrange("j (p c) -> p j c", c=512))

        # ---------------- Stage 6: out = O^T.T @ WO, hl-split --------------
        with ExitStack() as sctx:
            pp6 = sctx.enter_context(tc.tile_pool(name=f"ps6{rep}", bufs=4, space="PSUM"))
            outp = sctx.enter_context(tc.tile_pool(name=f"outp{rep}", bufs=2))
            acc = outp.tile([P, 4, 2048], F32, tag="acc")
            for rt in range(4):
                for ng in range(4):
                    ps = pp6.tile([P, 512], F32)
                    for i, h in enumerate(range(0, 16, 2)):
                        nc.tensor.matmul(
                            ps, lhsT=otf[:, h, rt * 128:(rt + 1) * 128],
                            rhs=wo_sb[:, h, ng * 512:(ng + 1) * 512],
                            start=(i == 0), stop=(i == 7))
                    nc.scalar.copy(acc[:, rt, ng * 512:(ng + 1) * 512], ps)
            for rt in range(4):
                out_t = outp.tile([P, 2048], BF16, tag="outt")
                for ng in range(4):
                    ps = pp6.tile([P, 512], F32)
                    for i, h in enumerate(range(1, 16, 2)):
                        nc.tensor.matmul(
                            ps, lhsT=otf[:, h, rt * 128:(rt + 1) * 128],
                            rhs=wo_sb[:, h, ng * 512:(ng + 1) * 512],
                            start=(i == 0), stop=(i == 7))
                    nc.vector.tensor_add(
                        out_t[:, ng * 512:(ng + 1) * 512],
                        acc[:, rt, ng * 512:(ng + 1) * 512], ps)
                    # Act ring: the SP ring carries the next rep's stage-1
                    # loads, which must not queue behind the output store
                    nc.scalar.dma_start(
                        out=OUT[rt * 128:(rt + 1) * 128, ng * 512:(ng + 1) * 512],
                        in_=out_t[:, ng * 512:(ng + 1) * 512])


# ---------------------------------------------------------------------------
# Host-side prep
# ---------------------------------------------------------------------------

def _bf(a):
    return np.asarray(a, dtype=np.float32).astype(BF16NP)


def _prep_weights(wq_a, q_norm_w, wq_b, wkv_a, kv_norm_w, wkv_b, wo,
                  freqs_cos, freqs_sin):
    wkv_a_lat = wkv_a[:, :KV_LORA]
    wkv_a_rope = wkv_a[:, KV_LORA:]
    wkv_a_rope = np.concatenate([wkv_a_rope[:, 0::2], wkv_a_rope[:, 1::2]], axis=1)
    WAh = np.concatenate([wq_a, wkv_a_lat, wkv_a_rope], axis=1)      # [2048, 2112]

    wqb = (wq_b * SCALE) * q_norm_w[:, None]
    wqb = wqb.reshape(Q_LORA, NH, 192)
    rope_e = wqb[:, :, NOPE + 0::2].reshape(Q_LORA, NH * 32)
    rope_o = wqb[:, :, NOPE + 1::2].reshape(Q_LORA, NH * 32)
    nope = wqb[:, :, :NOPE]                                           # [QL, 16, 128]
    nope_even = nope[:, 0::2, :].reshape(Q_LORA, 8 * NOPE)
    nope_odd = nope[:, 1::2, :].reshape(Q_LORA, 8 * NOPE)
    # column order must match _qcol(): [rope_e | rope_o | even nope | odd]
    WQBh = np.concatenate([rope_e, rope_o, nope_even, nope_odd], axis=1)

    WKVBh = wkv_b * kv_norm_w[:, None]                                # [512, 4096]
    pos = np.arange(R) % S
    COS = freqs_cos[pos].astype(np.float32)                           # [4096, 32]
    SIN = freqs_sin[pos].astype(np.float32)
    p = np.arange(128)[:, None]
    c = np.arange(512)[None, :]
    MASK = np.stack([(c - 128 * m - p >= 0) for m in range(4)],
                    axis=1).astype(np.float32)                        # [128, 4, 512]
    return dict(WA=_bf(WAh), WQB=_bf(WQBh), WKVB=_bf(WKVBh), WO=_bf(wo),
                COS=COS, SIN=SIN, MASK=_bf(MASK.reshape(128, 2048)))


def _prep_in_maps(inputs):
    x = np.asarray(inputs["x"], dtype=np.float32).reshape(R, 2048)
    W = _prep_weights(
        np.asarray(inputs["wq_a"]), np.asarray(inputs["q_norm_w"]),
        np.asarray(inputs["wq_b"]), np.asarray(inputs["wkv_a"]),
        np.asarray(inputs["kv_norm_w"]), np.asarray(inputs["wkv_b"]),
        np.asarray(inputs["wo"]),
        np.asarray(inputs["freqs_cos"]), np.asarray(inputs["freqs_sin"]))
    in_maps = []
    for c in range(NC_):
        rows = slice(c * LR, (c + 1) * LR)
        in_maps.append({
            "xt": np.ascontiguousarray(x[rows].T).astype(BF16NP),
            "wa": W["WA"], "wqb": W["WQB"],
            # this core's 2 heads (2c, 2c+1): cols [c*512, (c+1)*512)
            "wkvb": np.ascontiguousarray(W["WKVB"][:, c * 512:(c + 1) * 512]),
            "wo": W["WO"],
            "cost": np.ascontiguousarray(np.tile(W["COS"][rows].T, (4, 1))),
            "sint": np.ascontiguousarray(np.tile(W["SIN"][rows].T, (4, 1))),
            "masks": W["MASK"],
        })
    return in_maps


prep_in_maps = _prep_in_maps

_NC_CACHE = []


def _get_nc():
    if not _NC_CACHE:
        _NC_CACHE.append(build_kernel())
    return _NC_CACHE[0]


def kernel(**inputs) -> np.ndarray:
    in_maps = _prep_in_maps(inputs)
    nc = _get_nc()
    res = run_bass_kernel_spmd(nc, in_maps, core_ids=list(range(NC_)))
    outs = [np.asarray(res.results[c]["out"]).astype(np.float32)
            for c in range(NC_)]
    return np.concatenate(outs, axis=0).reshape(B, S, 2048)
